# revision 2
# baseline (speedup 1.0000x reference)
"""BayesGAT layer (eval mode) on 8 Trainium2 NeuronCores.

Math (identical to the reference, with dead code removed):
    qk[n]   = z[n] @ (tau * Wq @ Wk^T) + tau * bq @ Wk^T
    e[n,j]  = qk[n] . z[src[n,j]]          (+ const/row term from bk, which
                                            cancels in the softmax -> dropped)
    alpha   = softmax_j(e)
    h[n]    = (sum_j alpha[n,j] * z[src[n,j]]) @ Wv + bv

Distribution: dst nodes sharded 8 ways (6250/core, padded to 49 tiles of
128).  The z table (bf16, with a zeros row inserted at row 25000) is
replicated in every core's HBM; per-edge rows are fetched with the SWDGE
dma_gather instruction (int16 indices) striped over 4 SWDGE queues.  Since
int16 can't span 50001 rows, each tile does two gather passes — one from
the table base (covers rows <= 32767) and one from an offset view (rows >=
32768) — with off-range slots pointed at the zeros row; the two passes are
summed on-chip.  Scores run on the vector engine (bf16), softmax on
DVE+ACT (fp32), aggregation as 16 alpha-scaled PE transposes accumulated
in PSUM, and the output projection as one PE matmul per tile producing
h^T, which the host transposes back.
"""
import sys

sys.path.insert(0, "/opt/trn_rl_repo")

import numpy as np
import ml_dtypes

import concourse.bacc as bacc
import concourse.bass as bass
import concourse.tile as tile
from concourse import mybir

N = 50000
K = 16
D = 128
P = 128
NCORES = 8
NPC = N // NCORES            # 6250 dst nodes per core
TILES = (NPC + P - 1) // P   # 49
NPAD = TILES * P             # 6272
TAU = 1.0 / np.sqrt(D)
ZROW = 25000                 # zeros row position inside z_ext
HIBASE = 18000               # base row of the "hi" gather view
NQ = 4                       # SWDGE queues

_f32 = mybir.dt.float32
_bf16 = mybir.dt.bfloat16
_i16 = mybir.dt.int16

_cache = {}


# --------------------------------------------------------------------------
# Workaround: this container's walrus accepts only ONE sync-wait command per
# instruction.  Hoist extra waits of multi-wait instructions onto preceding
# single-wait NOPs on the same engine (sequencers run per-engine program
# order, so this is semantically identical).
_ctr = [0]


def _split_multiwait(nc):
    for f in nc.m.functions:
        for bb in f.blocks:
            insts = bb.instructions
            if not any(
                i.sync_info and i.sync_info.on_wait and len(i.sync_info.on_wait) > 1
                for i in insts
            ):
                continue
            out = []
            for inst in insts:
                si = inst.sync_info
                if si and si.on_wait and len(si.on_wait) > 1:
                    waits = list(si.on_wait)
                    for w in waits[:-1]:
                        _ctr[0] += 1
                        nop = mybir.InstNoOp(
                            name=f"I-waitsplit-{_ctr[0]}", ins=[], outs=[]
                        )
                        nop.engine = inst.engine
                        nop.sync_info = mybir.SyncInfo(on_wait=[w], on_update=[])
                        out.append(nop)
                    si.on_wait = [waits[-1]]
                    inst.sync_info = si
                out.append(inst)
            bb.instructions = out


# --------------------------------------------------------------------------
# Minimal PJRT runner (axon): compile once, device-resident inputs.
class _Runner:
    def __init__(self, nc, n_cores):
        import jax
        from concourse.bass2jax import install_neuronx_cc_hook

        install_neuronx_cc_hook()
        self.jax = jax
        self.nc = nc
        self.n_cores = n_cores
        self.in_names, self.out_names, self.out_avals, self.partition_name = (
            self._io_names(nc)
        )
        self.devices = jax.devices()[:n_cores]
        self._fn = None

    @staticmethod
    def _io_names(nc):
        import jax

        in_names, out_names, out_avals = [], [], []
        pname = nc.partition_id_tensor.name if nc.partition_id_tensor else None
        for alloc in nc.m.functions[0].allocations:
            if not isinstance(alloc, mybir.MemoryLocationSet):
                continue
            name = alloc.memorylocations[0].name
            if alloc.kind == "ExternalInput":
                if name != pname:
                    in_names.append(name)
            elif alloc.kind == "ExternalOutput":
                out_names.append(name)
                out_avals.append(
                    jax.core.ShapedArray(
                        tuple(alloc.tensor_shape), mybir.dt.np(alloc.dtype)
                    )
                )
        return in_names, out_names, out_avals, pname

    def _build_fn(self):
        import jax
        from jax.sharding import Mesh, PartitionSpec
        from jax.experimental.shard_map import shard_map
        from concourse.bass2jax import _bass_exec_p, partition_id_tensor

        nc = self.nc
        n_params = len(self.in_names)
        n_outs = len(self.out_names)
        all_in_names = list(self.in_names) + list(self.out_names)
        if self.partition_name is not None:
            all_in_names.append(self.partition_name)
        out_avals = tuple(self.out_avals)
        out_names = tuple(self.out_names)
        pname = self.partition_name

        def _body(*args):
            operands = list(args)
            if pname is not None:
                operands.append(partition_id_tensor())
            return tuple(
                _bass_exec_p.bind(
                    *operands,
                    out_avals=out_avals,
                    in_names=tuple(all_in_names),
                    out_names=out_names,
                    lowering_input_output_aliases=(),
                    sim_require_finite=True,
                    sim_require_nnan=True,
                    nc=nc,
                )
            )

        donate = tuple(range(n_params, n_params + n_outs))
        if self.n_cores == 1:
            return jax.jit(_body, donate_argnums=donate, keep_unused=True)
        mesh = Mesh(np.asarray(self.devices), ("core",))
        in_specs = (PartitionSpec("core"),) * (n_params + n_outs)
        out_specs = (PartitionSpec("core"),) * n_outs
        return jax.jit(
            shard_map(
                _body, mesh=mesh, in_specs=in_specs, out_specs=out_specs,
                check_rep=False,
            ),
            donate_argnums=donate,
            keep_unused=True,
        )

    def run(self, in_maps):
        jax = self.jax
        if self._fn is None:
            self._fn = self._build_fn()
        args = []
        for name in self.in_names:
            per = [np.asarray(in_maps[c][name]) for c in range(self.n_cores)]
            args.append(per[0] if self.n_cores == 1 else np.concatenate(per, 0))
        mult = 1 if self.n_cores == 1 else self.n_cores
        zeros = [
            np.zeros((mult * a.shape[0], *a.shape[1:]), a.dtype)
            for a in self.out_avals
        ]
        outs = self._fn(*args, *zeros)
        jax.block_until_ready(outs)
        res = []
        for c in range(self.n_cores):
            m = {}
            for i, name in enumerate(self.out_names):
                a = np.asarray(outs[i])
                if self.n_cores > 1:
                    a = a.reshape(self.n_cores, *self.out_avals[i].shape)[c]
                m[name] = a
            res.append(m)
        return res


# --------------------------------------------------------------------------
def _build(with_bq):
    nc = bacc.Bacc(
        "TRN2", target_bir_lowering=False, debug=False, num_swdge_queues=NQ
    )
    z_ext = nc.dram_tensor("z_ext", [N + 1, D], _bf16, kind="ExternalInput")
    zcT = nc.dram_tensor("zcT", [P, NPAD], _bf16, kind="ExternalInput")
    idx_lo = nc.dram_tensor("idx_lo", [P, TILES * 128], _i16, kind="ExternalInput")
    idx_hi = nc.dram_tensor("idx_hi", [P, TILES * 128], _i16, kind="ExternalInput")
    mt_d = nc.dram_tensor("Mt", [P, D], _bf16, kind="ExternalInput")
    wv_d = nc.dram_tensor("Wv", [P, D], _bf16, kind="ExternalInput")
    id_d = nc.dram_tensor("ident", [P, P], _bf16, kind="ExternalInput")
    bv_d = nc.dram_tensor("bv", [P, 1], _f32, kind="ExternalInput")
    if with_bq:
        m0_d = nc.dram_tensor("m0rep", [P, D], _f32, kind="ExternalInput")
    hT_d = nc.dram_tensor("hT", [P, NPAD], _f32, kind="ExternalOutput")
    al_d = nc.dram_tensor("alpha", [NPAD, K], _f32, kind="ExternalOutput")

    with tile.TileContext(nc) as tc:
        with (
            tc.tile_pool(name="const", bufs=1) as cpool,
            tc.tile_pool(name="glo", bufs=3) as glop,
            tc.tile_pool(name="ghi", bufs=3) as ghip,
            tc.tile_pool(name="g", bufs=3) as gp,
            tc.tile_pool(name="scr", bufs=2) as scrp,
            tc.tile_pool(name="sg", bufs=2) as sgp,
            tc.tile_pool(name="qk", bufs=3) as qkp,
            tc.tile_pool(name="small", bufs=6) as smp,
            tc.tile_pool(name="outs", bufs=3) as outp,
            tc.tile_pool(name="ps_qk", bufs=2, space="PSUM") as ps_qk,
            tc.tile_pool(name="ps_agg", bufs=2, space="PSUM") as ps_agg,
            tc.tile_pool(name="ps_ht", bufs=2, space="PSUM") as ps_ht,
        ):
            mt = cpool.tile([P, D], _bf16)
            nc.sync.dma_start(out=mt[:], in_=mt_d[:])
            wv = cpool.tile([P, D], _bf16)
            nc.sync.dma_start(out=wv[:], in_=wv_d[:])
            ident = cpool.tile([P, P], _bf16)
            nc.sync.dma_start(out=ident[:], in_=id_d[:])
            bv = cpool.tile([P, 1], _f32)
            nc.sync.dma_start(out=bv[:], in_=bv_d[:])
            if with_bq:
                m0 = cpool.tile([P, D], _f32)
                nc.sync.dma_start(out=m0[:], in_=m0_d[:])
            zct = cpool.tile([P, NPAD], _bf16)
            nc.sync.dma_start(out=zct[:], in_=zcT[:])
            ilo = cpool.tile([P, TILES * 128], _i16)
            nc.sync.dma_start(out=ilo[:], in_=idx_lo[:])
            ihi = cpool.tile([P, TILES * 128], _i16)
            nc.sync.dma_start(out=ihi[:], in_=idx_hi[:])

            z_lo = z_ext[:, :]
            z_hi = z_ext[HIBASE:, :]

            qi = 0
            for t in range(TILES):
                glo = glop.tile([P, K, D], _bf16)
                ghi = ghip.tile([P, K, D], _bf16)
                for h in range(2):
                    nc.gpsimd.dma_gather(
                        out_ap=glo[:, h * 8:(h + 1) * 8, :],
                        in_ap=z_lo,
                        idxs_ap=ilo[:, t * 128 + h * 64: t * 128 + (h + 1) * 64],
                        num_idxs=1024, num_idxs_reg=1024, elem_size=D,
                        queue_num=qi % NQ,
                    )
                    qi += 1
                for h in range(2):
                    nc.gpsimd.dma_gather(
                        out_ap=ghi[:, h * 8:(h + 1) * 8, :],
                        in_ap=z_hi,
                        idxs_ap=ihi[:, t * 128 + h * 64: t * 128 + (h + 1) * 64],
                        num_idxs=1024, num_idxs_reg=1024, elem_size=D,
                        queue_num=qi % NQ,
                    )
                    qi += 1

                # combine the two gather passes (one of each pair is zeros)
                g = gp.tile([P, K, D], _bf16)
                nc.vector.tensor_tensor(
                    out=g[:, :, :], in0=glo[:, :, :], in1=ghi[:, :, :],
                    op=mybir.AluOpType.add,
                )

                # qk = z_c @ Mt  (row-major: lhsT = zcT slice)
                qk_ps = ps_qk.tile([P, D], _f32)
                nc.tensor.matmul(
                    out=qk_ps[:], lhsT=zct[:, t * P:(t + 1) * P], rhs=mt[:],
                    start=True, stop=True,
                )
                qk = qkp.tile([P, D], _bf16)
                if with_bq:
                    nc.vector.tensor_tensor(
                        out=qk[:], in0=qk_ps[:], in1=m0[:],
                        op=mybir.AluOpType.add,
                    )
                else:
                    nc.scalar.activation(
                        out=qk[:], in_=qk_ps[:],
                        func=mybir.ActivationFunctionType.Copy,
                    )

                # e[n,j] = sum_d g[n,j,d] * qk[n,d]
                qk_b = bass.AP(
                    tensor=qk[:].tensor, offset=qk[:].offset,
                    ap=[qk[:].ap[0], [0, K], qk[:].ap[1]],
                )
                scr = scrp.tile([P, K, D], _bf16)
                nc.vector.tensor_tensor(
                    out=scr[:, :, :], in0=g[:, :, :], in1=qk_b,
                    op=mybir.AluOpType.mult,
                )
                e = smp.tile([P, K], _f32)
                nc.vector.tensor_reduce(
                    out=e[:], in_=scr[:, :, :], axis=mybir.AxisListType.X,
                    op=mybir.AluOpType.add,
                )

                # softmax over j (fp32)
                mx = smp.tile([P, 1], _f32)
                nc.vector.tensor_reduce(
                    out=mx[:], in_=e[:], axis=mybir.AxisListType.X,
                    op=mybir.AluOpType.max,
                )
                es = smp.tile([P, K], _f32)
                nc.vector.tensor_scalar_sub(out=es[:], in0=e[:], scalar1=mx[:])
                pex = smp.tile([P, K], _f32)
                nc.scalar.activation(
                    out=pex[:], in_=es[:], func=mybir.ActivationFunctionType.Exp,
                )
                s = smp.tile([P, 1], _f32)
                nc.vector.tensor_reduce(
                    out=s[:], in_=pex[:], axis=mybir.AxisListType.X,
                    op=mybir.AluOpType.add,
                )
                rinv = smp.tile([P, 1], _f32)
                nc.vector.reciprocal(out=rinv[:], in_=s[:])
                alph = outp.tile([P, K], _f32)
                nc.vector.tensor_scalar_mul(out=alph[:], in0=pex[:], scalar1=rinv[:])

                # aggregation: aggT = sum_j (alpha_j * G_j)^T  via PE transposes
                sg = sgp.tile([P, K, D], _bf16)
                for j in range(K):
                    nc.vector.tensor_scalar_mul(
                        out=sg[:, j, :], in0=g[:, j, :], scalar1=alph[:, j:j + 1],
                    )
                agg_ps = ps_agg.tile([P, P], _f32)
                for j in range(K):
                    nc.tensor.matmul(
                        out=agg_ps[:], lhsT=sg[:, j, :], rhs=ident[:],
                        start=(j == 0), stop=(j == K - 1),
                    )
                aggsb = qkp.tile([P, P], _bf16)
                nc.scalar.activation(
                    out=aggsb[:], in_=agg_ps[:],
                    func=mybir.ActivationFunctionType.Copy,
                )

                # hT = Wv^T @ aggT  (+ bv per-partition)
                ht_ps = ps_ht.tile([P, P], _f32)
                nc.tensor.matmul(
                    out=ht_ps[:], lhsT=wv[:], rhs=aggsb[:], start=True, stop=True,
                )
                ht = outp.tile([P, P], _f32)
                nc.scalar.activation(
                    out=ht[:], in_=ht_ps[:],
                    func=mybir.ActivationFunctionType.Identity, bias=bv[:],
                )

                nc.sync.dma_start(out=hT_d[:, t * P:(t + 1) * P], in_=ht[:])
                nc.sync.dma_start(out=al_d[t * P:(t + 1) * P, :], in_=alph[:])

    nc.compile()
    _split_multiwait(nc)
    return nc


def _wrap_idx(arr):
    """[TILES, 2048] edge-ordered (j-major) int16 -> [128, TILES*128] wrapped
    layout: per 1024-idx instruction, idx i lives at [i%16 (+16r), i//16]."""
    T = arr.shape[0]
    a = arr.reshape(T, 2, 64, 16).transpose(0, 1, 3, 2)     # [T, 2, 16, 64]
    a = np.tile(a, (1, 1, 8, 1))                            # [T, 2, 128, 64]
    return np.ascontiguousarray(
        a.transpose(2, 0, 1, 3).reshape(P, T * 128)
    )


def kernel(z, src, Wq, bq, Wk, bk, Wv, bv, Ws1, bs1, Ws2, bs2):
    z = np.asarray(z, dtype=np.float32)
    src = np.asarray(src).astype(np.int64)
    Wq = np.asarray(Wq, dtype=np.float32)
    Wk = np.asarray(Wk, dtype=np.float32)
    Wv_ = np.asarray(Wv, dtype=np.float32)
    bq = np.asarray(bq, dtype=np.float32)
    bv_ = np.asarray(bv, dtype=np.float32)

    bf = ml_dtypes.bfloat16
    Mt = (TAU * (Wq @ Wk.T)).astype(bf)
    m0 = (TAU * (bq @ Wk.T)).astype(np.float32)          # [D]
    with_bq = bool(np.any(m0))

    # gather table with a zeros row spliced in at ZROW
    z_ext = np.empty((N + 1, D), dtype=bf)
    z_ext[:ZROW] = z[:ZROW].astype(bf)
    z_ext[ZROW] = 0
    z_ext[ZROW + 1:] = z[ZROW:].astype(bf)

    key = with_bq
    if key not in _cache:
        nc = _build(with_bq)
        _cache[key] = (nc, _Runner(nc, NCORES))
    nc, runner = _cache[key]

    ident = np.eye(P, dtype=bf)
    bv_col = bv_.reshape(P, 1)
    Wv_b = Wv_.astype(bf)
    m0rep = np.tile(m0.reshape(1, D), (P, 1)).astype(np.float32)

    mapped = src + (src >= ZROW)                         # [N, K] rows in z_ext
    in_maps = []
    for c in range(NCORES):
        rows = np.arange(c * NPC, (c + 1) * NPC)
        zc = np.zeros((NPAD, D), np.float32)
        zc[:NPC] = z[rows]
        zcT = np.ascontiguousarray(zc.T).astype(bf)      # [128, NPAD]

        mc = np.zeros((NPAD, K), np.int64)
        mc[:NPC] = mapped[rows]
        lo = np.where(mc <= 32767, mc, ZROW).astype(np.int16)
        hi = np.where(mc >= 32768, mc - HIBASE, ZROW - HIBASE).astype(np.int16)
        # edge order per tile: slot i = j*128 + n  (j-major)
        lo_t = lo.reshape(TILES, P, K).transpose(0, 2, 1).reshape(TILES, K * P)
        hi_t = hi.reshape(TILES, P, K).transpose(0, 2, 1).reshape(TILES, K * P)

        m_ = {
            "z_ext": z_ext,
            "zcT": zcT,
            "idx_lo": _wrap_idx(lo_t),
            "idx_hi": _wrap_idx(hi_t),
            "Mt": Mt,
            "Wv": Wv_b,
            "ident": ident,
            "bv": bv_col,
        }
        if with_bq:
            m_["m0rep"] = m0rep
        in_maps.append(m_)

    res = runner.run(in_maps)

    h = np.empty((N, D), np.float32)
    alpha = np.empty((N, K), np.float32)
    for c in range(NCORES):
        h[c * NPC:(c + 1) * NPC] = res[c]["hT"][:, :NPC].T
        alpha[c * NPC:(c + 1) * NPC] = res[c]["alpha"][:NPC]
    return h, alpha


# revision 4
# speedup vs baseline: 15.2880x; 15.2880x over previous
"""BayesGAT layer (eval mode) on 8 Trainium2 NeuronCores.

Math (identical to the reference, with dead code removed):
    qk[n]   = z[n] @ (tau * Wq @ Wk^T) + tau * bq @ Wk^T
    e[n,j]  = qk[n] . z[src[n,j]]          (+ const/row term from bk, which
                                            cancels in the softmax -> dropped)
    alpha   = softmax_j(e)
    h[n]    = (sum_j alpha[n,j] * z[src[n,j]]) @ Wv + bv

Distribution: dst nodes sharded 8 ways (6250/core, padded to 49 tiles of
128).  The z table (bf16, with a zeros row inserted at row 25000) is
replicated in every core's HBM; per-edge rows are fetched with the SWDGE
dma_gather instruction (int16 indices) striped over 4 SWDGE queues.  Since
int16 can't span 50001 rows, each tile does two gather passes — one from
the table base (covers rows <= 32767) and one from an offset view (rows >=
32768) — with off-range slots pointed at the zeros row; the two passes are
summed on-chip.  Scores run on the vector engine (bf16), softmax on
DVE+ACT (fp32), aggregation as 16 alpha-scaled PE transposes accumulated
in PSUM, and the output projection as one PE matmul per tile producing
h^T, which the host transposes back.
"""
import sys

sys.path.insert(0, "/opt/trn_rl_repo")

import numpy as np
import ml_dtypes

import concourse.bacc as bacc
import concourse.bass as bass
import concourse.tile as tile
from concourse import mybir

N = 50000
K = 16
D = 128
P = 128
NCORES = 8
NPC = N // NCORES            # 6250 dst nodes per core
TILES = (NPC + P - 1) // P   # 49
NPAD = TILES * P             # 6272
TAU = 1.0 / np.sqrt(D)
ZROW = 25000                 # zeros row position inside z_ext
HIBASE = 18000               # base row of the "hi" gather view
NQ = 4                       # SWDGE queues

_f32 = mybir.dt.float32
_bf16 = mybir.dt.bfloat16
_i16 = mybir.dt.int16

_cache = {}


# --------------------------------------------------------------------------
# Workaround: this container's walrus accepts only ONE sync-wait command per
# instruction.  Hoist extra waits of multi-wait instructions onto preceding
# single-wait NOPs on the same engine (sequencers run per-engine program
# order, so this is semantically identical).
_ctr = [0]


def _split_multiwait(nc):
    for f in nc.m.functions:
        for bb in f.blocks:
            insts = bb.instructions
            if not any(
                i.sync_info and i.sync_info.on_wait and len(i.sync_info.on_wait) > 1
                for i in insts
            ):
                continue
            out = []
            for inst in insts:
                si = inst.sync_info
                if si and si.on_wait and len(si.on_wait) > 1:
                    waits = list(si.on_wait)
                    for w in waits[:-1]:
                        _ctr[0] += 1
                        nop = mybir.InstNoOp(
                            name=f"I-waitsplit-{_ctr[0]}", ins=[], outs=[]
                        )
                        nop.engine = inst.engine
                        nop.sync_info = mybir.SyncInfo(on_wait=[w], on_update=[])
                        out.append(nop)
                    si.on_wait = [waits[-1]]
                    inst.sync_info = si
                out.append(inst)
            bb.instructions = out


# --------------------------------------------------------------------------
# Minimal PJRT runner (axon): compile once, device-resident inputs.
class _Runner:
    def __init__(self, nc, n_cores):
        import jax
        from concourse.bass2jax import install_neuronx_cc_hook

        install_neuronx_cc_hook()
        self.jax = jax
        self.nc = nc
        self.n_cores = n_cores
        self.in_names, self.out_names, self.out_avals, self.partition_name = (
            self._io_names(nc)
        )
        self.devices = jax.devices()[:n_cores]
        self._fn = None

    @staticmethod
    def _io_names(nc):
        import jax

        in_names, out_names, out_avals = [], [], []
        pname = nc.partition_id_tensor.name if nc.partition_id_tensor else None
        for alloc in nc.m.functions[0].allocations:
            if not isinstance(alloc, mybir.MemoryLocationSet):
                continue
            name = alloc.memorylocations[0].name
            if alloc.kind == "ExternalInput":
                if name != pname:
                    in_names.append(name)
            elif alloc.kind == "ExternalOutput":
                out_names.append(name)
                out_avals.append(
                    jax.core.ShapedArray(
                        tuple(alloc.tensor_shape), mybir.dt.np(alloc.dtype)
                    )
                )
        return in_names, out_names, out_avals, pname

    def _build_fn(self):
        import jax
        from jax.sharding import Mesh, PartitionSpec
        from jax.experimental.shard_map import shard_map
        from concourse.bass2jax import _bass_exec_p, partition_id_tensor

        nc = self.nc
        n_params = len(self.in_names)
        n_outs = len(self.out_names)
        all_in_names = list(self.in_names) + list(self.out_names)
        if self.partition_name is not None:
            all_in_names.append(self.partition_name)
        out_avals = tuple(self.out_avals)
        out_names = tuple(self.out_names)
        pname = self.partition_name

        def _body(*args):
            operands = list(args)
            if pname is not None:
                operands.append(partition_id_tensor())
            return tuple(
                _bass_exec_p.bind(
                    *operands,
                    out_avals=out_avals,
                    in_names=tuple(all_in_names),
                    out_names=out_names,
                    lowering_input_output_aliases=(),
                    sim_require_finite=True,
                    sim_require_nnan=True,
                    nc=nc,
                )
            )

        donate = tuple(range(n_params, n_params + n_outs))
        if self.n_cores == 1:
            return jax.jit(_body, donate_argnums=donate, keep_unused=True)
        mesh = Mesh(np.asarray(self.devices), ("core",))
        in_specs = (PartitionSpec("core"),) * (n_params + n_outs)
        out_specs = (PartitionSpec("core"),) * n_outs
        return jax.jit(
            shard_map(
                _body, mesh=mesh, in_specs=in_specs, out_specs=out_specs,
                check_rep=False,
            ),
            donate_argnums=donate,
            keep_unused=True,
        )

    def put_inputs(self, in_maps):
        import jax
        from jax.sharding import Mesh, PartitionSpec, NamedSharding

        args = []
        for name in self.in_names:
            per = [np.asarray(in_maps[c][name]) for c in range(self.n_cores)]
            args.append(per[0] if self.n_cores == 1 else np.concatenate(per, 0))
        if self.n_cores == 1:
            return [jax.device_put(a, self.devices[0]) for a in args]
        mesh = Mesh(np.asarray(self.devices), ("core",))
        sh = NamedSharding(mesh, PartitionSpec("core"))
        return [jax.device_put(a, sh) for a in args]

    def run_device(self, dev_args):
        jax = self.jax
        if self._fn is None:
            self._fn = self._build_fn()
        mult = 1 if self.n_cores == 1 else self.n_cores
        zeros = [
            np.zeros((mult * a.shape[0], *a.shape[1:]), a.dtype)
            for a in self.out_avals
        ]
        outs = self._fn(*dev_args, *zeros)
        jax.block_until_ready(outs)
        return outs

    def run(self, in_maps):
        outs = self.run_device(self.put_inputs(in_maps))
        res = []
        for c in range(self.n_cores):
            m = {}
            for i, name in enumerate(self.out_names):
                a = np.asarray(outs[i])
                if self.n_cores > 1:
                    a = a.reshape(self.n_cores, *self.out_avals[i].shape)[c]
                m[name] = a
            res.append(m)
        return res


# --------------------------------------------------------------------------
def _build(with_bq):
    nc = bacc.Bacc(
        "TRN2", target_bir_lowering=False, debug=False, num_swdge_queues=NQ
    )
    z_ext = nc.dram_tensor("z_ext", [N + 1, D], _bf16, kind="ExternalInput")
    zcT = nc.dram_tensor("zcT", [P, NPAD], _bf16, kind="ExternalInput")
    idx_lo = nc.dram_tensor("idx_lo", [P, TILES * 128], _i16, kind="ExternalInput")
    idx_hi = nc.dram_tensor("idx_hi", [P, TILES * 128], _i16, kind="ExternalInput")
    mt_d = nc.dram_tensor("Mt", [P, D], _bf16, kind="ExternalInput")
    wv_d = nc.dram_tensor("Wv", [P, D], _bf16, kind="ExternalInput")
    id_d = nc.dram_tensor("ident", [P, P], _bf16, kind="ExternalInput")
    bv_d = nc.dram_tensor("bv", [P, 1], _f32, kind="ExternalInput")
    if with_bq:
        m0_d = nc.dram_tensor("m0rep", [P, D], _f32, kind="ExternalInput")
    hT_d = nc.dram_tensor("hT", [P, NPAD], _f32, kind="ExternalOutput")
    al_d = nc.dram_tensor("alpha", [NPAD, K], _f32, kind="ExternalOutput")

    with tile.TileContext(nc) as tc:
        with (
            tc.tile_pool(name="const", bufs=1) as cpool,
            tc.tile_pool(name="glo", bufs=3) as glop,
            tc.tile_pool(name="ghi", bufs=3) as ghip,
            tc.tile_pool(name="g", bufs=3) as gp,
            tc.tile_pool(name="scr", bufs=2) as scrp,
            tc.tile_pool(name="sg", bufs=2) as sgp,
            tc.tile_pool(name="qk", bufs=3) as qkp,
            tc.tile_pool(name="small", bufs=6) as smp,
            tc.tile_pool(name="outs", bufs=3) as outp,
            tc.tile_pool(name="ps_qk", bufs=2, space="PSUM") as ps_qk,
            tc.tile_pool(name="ps_agg", bufs=2, space="PSUM") as ps_agg,
            tc.tile_pool(name="ps_ht", bufs=2, space="PSUM") as ps_ht,
        ):
            mt = cpool.tile([P, D], _bf16)
            nc.sync.dma_start(out=mt[:], in_=mt_d[:])
            wv = cpool.tile([P, D], _bf16)
            nc.sync.dma_start(out=wv[:], in_=wv_d[:])
            ident = cpool.tile([P, P], _bf16)
            nc.sync.dma_start(out=ident[:], in_=id_d[:])
            bv = cpool.tile([P, 1], _f32)
            nc.sync.dma_start(out=bv[:], in_=bv_d[:])
            if with_bq:
                m0 = cpool.tile([P, D], _f32)
                nc.sync.dma_start(out=m0[:], in_=m0_d[:])
            zct = cpool.tile([P, NPAD], _bf16)
            nc.sync.dma_start(out=zct[:], in_=zcT[:])
            ilo = cpool.tile([P, TILES * 128], _i16)
            nc.sync.dma_start(out=ilo[:], in_=idx_lo[:])
            ihi = cpool.tile([P, TILES * 128], _i16)
            nc.sync.dma_start(out=ihi[:], in_=idx_hi[:])

            z_lo = z_ext[:, :]
            z_hi = z_ext[HIBASE:, :]

            qi = 0
            for t in range(TILES):
                glo = glop.tile([P, K, D], _bf16)
                ghi = ghip.tile([P, K, D], _bf16)
                for h in range(2):
                    nc.gpsimd.dma_gather(
                        out_ap=glo[:, h * 8:(h + 1) * 8, :],
                        in_ap=z_lo,
                        idxs_ap=ilo[:, t * 128 + h * 64: t * 128 + (h + 1) * 64],
                        num_idxs=1024, num_idxs_reg=1024, elem_size=D,
                        queue_num=qi % NQ,
                    )
                    qi += 1
                for h in range(2):
                    nc.gpsimd.dma_gather(
                        out_ap=ghi[:, h * 8:(h + 1) * 8, :],
                        in_ap=z_hi,
                        idxs_ap=ihi[:, t * 128 + h * 64: t * 128 + (h + 1) * 64],
                        num_idxs=1024, num_idxs_reg=1024, elem_size=D,
                        queue_num=qi % NQ,
                    )
                    qi += 1

                # combine the two gather passes (one of each pair is zeros)
                g = gp.tile([P, K, D], _bf16)
                nc.vector.tensor_tensor(
                    out=g[:, :, :], in0=glo[:, :, :], in1=ghi[:, :, :],
                    op=mybir.AluOpType.add,
                )

                # qk = z_c @ Mt  (row-major: lhsT = zcT slice)
                qk_ps = ps_qk.tile([P, D], _f32)
                nc.tensor.matmul(
                    out=qk_ps[:], lhsT=zct[:, t * P:(t + 1) * P], rhs=mt[:],
                    start=True, stop=True,
                )
                qk = qkp.tile([P, D], _bf16)
                if with_bq:
                    nc.vector.tensor_tensor(
                        out=qk[:], in0=qk_ps[:], in1=m0[:],
                        op=mybir.AluOpType.add,
                    )
                else:
                    nc.scalar.activation(
                        out=qk[:], in_=qk_ps[:],
                        func=mybir.ActivationFunctionType.Copy,
                    )

                # e[n,j] = sum_d g[n,j,d] * qk[n,d]
                qk_b = bass.AP(
                    tensor=qk[:].tensor, offset=qk[:].offset,
                    ap=[qk[:].ap[0], [0, K], qk[:].ap[1]],
                )
                scr = scrp.tile([P, K, D], _bf16)
                nc.vector.tensor_tensor(
                    out=scr[:, :, :], in0=g[:, :, :], in1=qk_b,
                    op=mybir.AluOpType.mult,
                )
                e = smp.tile([P, K], _f32)
                nc.vector.tensor_reduce(
                    out=e[:], in_=scr[:, :, :], axis=mybir.AxisListType.X,
                    op=mybir.AluOpType.add,
                )

                # softmax over j (fp32)
                mx = smp.tile([P, 1], _f32)
                nc.vector.tensor_reduce(
                    out=mx[:], in_=e[:], axis=mybir.AxisListType.X,
                    op=mybir.AluOpType.max,
                )
                es = smp.tile([P, K], _f32)
                nc.vector.tensor_scalar_sub(out=es[:], in0=e[:], scalar1=mx[:])
                pex = smp.tile([P, K], _f32)
                nc.scalar.activation(
                    out=pex[:], in_=es[:], func=mybir.ActivationFunctionType.Exp,
                )
                s = smp.tile([P, 1], _f32)
                nc.vector.tensor_reduce(
                    out=s[:], in_=pex[:], axis=mybir.AxisListType.X,
                    op=mybir.AluOpType.add,
                )
                rinv = smp.tile([P, 1], _f32)
                nc.vector.reciprocal(out=rinv[:], in_=s[:])
                alph = outp.tile([P, K], _f32)
                nc.vector.tensor_scalar_mul(out=alph[:], in0=pex[:], scalar1=rinv[:])

                # aggregation: aggT = sum_j (alpha_j * G_j)^T  via PE transposes
                sg = sgp.tile([P, K, D], _bf16)
                for j in range(K):
                    nc.vector.tensor_scalar_mul(
                        out=sg[:, j, :], in0=g[:, j, :], scalar1=alph[:, j:j + 1],
                    )
                agg_ps = ps_agg.tile([P, P], _f32)
                for j in range(K):
                    nc.tensor.matmul(
                        out=agg_ps[:], lhsT=sg[:, j, :], rhs=ident[:],
                        start=(j == 0), stop=(j == K - 1),
                    )
                aggsb = qkp.tile([P, P], _bf16)
                nc.scalar.activation(
                    out=aggsb[:], in_=agg_ps[:],
                    func=mybir.ActivationFunctionType.Copy,
                )

                # hT = Wv^T @ aggT  (+ bv per-partition)
                ht_ps = ps_ht.tile([P, P], _f32)
                nc.tensor.matmul(
                    out=ht_ps[:], lhsT=wv[:], rhs=aggsb[:], start=True, stop=True,
                )
                ht = outp.tile([P, P], _f32)
                nc.scalar.activation(
                    out=ht[:], in_=ht_ps[:],
                    func=mybir.ActivationFunctionType.Identity, bias=bv[:],
                )

                nc.sync.dma_start(out=hT_d[:, t * P:(t + 1) * P], in_=ht[:])
                nc.sync.dma_start(out=al_d[t * P:(t + 1) * P, :], in_=alph[:])

    nc.compile()
    _split_multiwait(nc)
    return nc


def _wrap_idx(arr):
    """[TILES, 2048] edge-ordered (j-major) int16 -> [128, TILES*128] wrapped
    layout: per 1024-idx instruction, idx i lives at [i%16 (+16r), i//16]."""
    T = arr.shape[0]
    a = arr.reshape(T, 2, 64, 16).transpose(0, 1, 3, 2)     # [T, 2, 16, 64]
    a = np.tile(a, (1, 1, 8, 1))                            # [T, 2, 128, 64]
    return np.ascontiguousarray(
        a.transpose(2, 0, 1, 3).reshape(P, T * 128)
    )


def kernel(z, src, Wq, bq, Wk, bk, Wv, bv, Ws1, bs1, Ws2, bs2):
    z = np.asarray(z, dtype=np.float32)
    src = np.asarray(src).astype(np.int64)
    Wq = np.asarray(Wq, dtype=np.float32)
    Wk = np.asarray(Wk, dtype=np.float32)
    Wv_ = np.asarray(Wv, dtype=np.float32)
    bq = np.asarray(bq, dtype=np.float32)
    bv_ = np.asarray(bv, dtype=np.float32)

    bf = ml_dtypes.bfloat16
    Mt = (TAU * (Wq @ Wk.T)).astype(bf)
    m0 = (TAU * (bq @ Wk.T)).astype(np.float32)          # [D]
    with_bq = bool(np.any(m0))

    # gather table with a zeros row spliced in at ZROW
    z_ext = np.empty((N + 1, D), dtype=bf)
    z_ext[:ZROW] = z[:ZROW].astype(bf)
    z_ext[ZROW] = 0
    z_ext[ZROW + 1:] = z[ZROW:].astype(bf)

    key = with_bq
    if key not in _cache:
        nc = _build(with_bq)
        _cache[key] = (nc, _Runner(nc, NCORES))
    nc, runner = _cache[key]

    ident = np.eye(P, dtype=bf)
    bv_col = bv_.reshape(P, 1)
    Wv_b = Wv_.astype(bf)
    m0rep = np.tile(m0.reshape(1, D), (P, 1)).astype(np.float32)

    mapped = src + (src >= ZROW)                         # [N, K] rows in z_ext
    in_maps = []
    for c in range(NCORES):
        rows = np.arange(c * NPC, (c + 1) * NPC)
        zc = np.zeros((NPAD, D), np.float32)
        zc[:NPC] = z[rows]
        zcT = np.ascontiguousarray(zc.T).astype(bf)      # [128, NPAD]

        mc = np.zeros((NPAD, K), np.int64)
        mc[:NPC] = mapped[rows]
        lo = np.where(mc <= 32767, mc, ZROW).astype(np.int16)
        hi = np.where(mc >= 32768, mc - HIBASE, ZROW - HIBASE).astype(np.int16)
        # edge order per tile: slot i = j*128 + n  (j-major)
        lo_t = lo.reshape(TILES, P, K).transpose(0, 2, 1).reshape(TILES, K * P)
        hi_t = hi.reshape(TILES, P, K).transpose(0, 2, 1).reshape(TILES, K * P)

        m_ = {
            "z_ext": z_ext,
            "zcT": zcT,
            "idx_lo": _wrap_idx(lo_t),
            "idx_hi": _wrap_idx(hi_t),
            "Mt": Mt,
            "Wv": Wv_b,
            "ident": ident,
            "bv": bv_col,
        }
        if with_bq:
            m_["m0rep"] = m0rep
        in_maps.append(m_)

    kernel._last_runner = runner
    kernel._last_in_maps = in_maps
    res = runner.run(in_maps)

    h = np.empty((N, D), np.float32)
    alpha = np.empty((N, K), np.float32)
    for c in range(NCORES):
        h[c * NPC:(c + 1) * NPC] = res[c]["hT"][:, :NPC].T
        alpha[c * NPC:(c + 1) * NPC] = res[c]["alpha"][:NPC]
    return h, alpha


# revision 5
# speedup vs baseline: 130.8067x; 8.5562x over previous
"""BayesGAT layer (eval mode) on 8 Trainium2 NeuronCores.

Math (identical to the reference, with dead code removed):
    qk[n]   = z[n] @ (tau * Wq @ Wk^T) + tau * bq @ Wk^T
    e[n,j]  = qk[n] . z[src[n,j]]          (+ const/row term from bk, which
                                            cancels in the softmax -> dropped)
    alpha   = softmax_j(e)
    h[n]    = (sum_j alpha[n,j] * z[src[n,j]]) @ Wv + bv

Distribution: dst nodes sharded 8 ways (6250/core, padded to 49 tiles of
128).  The z table (bf16, with a zeros row inserted at row 25000) is
replicated in every core's HBM; per-edge rows are fetched with the SWDGE
dma_gather instruction (int16 indices) striped over 4 SWDGE queues.  Since
int16 can't span 50001 rows, each tile does two gather passes — one from
the table base (covers rows <= 32767) and one from an offset view (rows >=
32768) — with off-range slots pointed at the zeros row; the two passes are
summed on-chip.  Scores run on the vector engine (bf16), softmax on
DVE+ACT (fp32), aggregation as 16 alpha-scaled PE transposes accumulated
in PSUM, and the output projection as one PE matmul per tile producing
h^T, which the host transposes back.
"""
import sys

sys.path.insert(0, "/opt/trn_rl_repo")

import numpy as np
import ml_dtypes

import concourse.bacc as bacc
import concourse.bass as bass
import concourse.tile as tile
from concourse import mybir

N = 50000
K = 16
D = 128
P = 128
NCORES = 8
NPC = N // NCORES            # 6250 dst nodes per core
TILES = (NPC + P - 1) // P   # 49
NPAD = TILES * P             # 6272
TAU = 1.0 / np.sqrt(D)
ZROW = 25000                 # zeros row position inside z_ext
HIBASE = 18000               # base row of the "hi" gather view
NQ = 4                       # SWDGE queues

_f32 = mybir.dt.float32
_bf16 = mybir.dt.bfloat16
_i16 = mybir.dt.int16

_cache = {}


# --------------------------------------------------------------------------
# Workaround: this container's walrus accepts only ONE sync-wait command per
# instruction.  Hoist extra waits of multi-wait instructions onto preceding
# single-wait NOPs on the same engine (sequencers run per-engine program
# order, so this is semantically identical).
_ctr = [0]


def _split_multiwait(nc):
    for f in nc.m.functions:
        for bb in f.blocks:
            insts = bb.instructions
            if not any(
                i.sync_info and i.sync_info.on_wait and len(i.sync_info.on_wait) > 1
                for i in insts
            ):
                continue
            out = []
            for inst in insts:
                si = inst.sync_info
                if si and si.on_wait and len(si.on_wait) > 1:
                    waits = list(si.on_wait)
                    for w in waits[:-1]:
                        _ctr[0] += 1
                        nop = mybir.InstNoOp(
                            name=f"I-waitsplit-{_ctr[0]}", ins=[], outs=[]
                        )
                        nop.engine = inst.engine
                        nop.sync_info = mybir.SyncInfo(on_wait=[w], on_update=[])
                        out.append(nop)
                    si.on_wait = [waits[-1]]
                    inst.sync_info = si
                out.append(inst)
            bb.instructions = out


# --------------------------------------------------------------------------
# Minimal PJRT runner (axon): compile once, device-resident inputs.
class _Runner:
    def __init__(self, nc, n_cores):
        import jax
        from concourse.bass2jax import install_neuronx_cc_hook

        install_neuronx_cc_hook()
        self.jax = jax
        self.nc = nc
        self.n_cores = n_cores
        self.in_names, self.out_names, self.out_avals, self.partition_name = (
            self._io_names(nc)
        )
        self.devices = jax.devices()[:n_cores]
        self._fn = None

    @staticmethod
    def _io_names(nc):
        import jax

        in_names, out_names, out_avals = [], [], []
        pname = nc.partition_id_tensor.name if nc.partition_id_tensor else None
        for alloc in nc.m.functions[0].allocations:
            if not isinstance(alloc, mybir.MemoryLocationSet):
                continue
            name = alloc.memorylocations[0].name
            if alloc.kind == "ExternalInput":
                if name != pname:
                    in_names.append(name)
            elif alloc.kind == "ExternalOutput":
                out_names.append(name)
                out_avals.append(
                    jax.core.ShapedArray(
                        tuple(alloc.tensor_shape), mybir.dt.np(alloc.dtype)
                    )
                )
        return in_names, out_names, out_avals, pname

    def _build_fn(self):
        import jax
        from jax.sharding import Mesh, PartitionSpec
        from jax.experimental.shard_map import shard_map
        from concourse.bass2jax import _bass_exec_p, partition_id_tensor

        nc = self.nc
        n_params = len(self.in_names)
        n_outs = len(self.out_names)
        all_in_names = list(self.in_names) + list(self.out_names)
        if self.partition_name is not None:
            all_in_names.append(self.partition_name)
        out_avals = tuple(self.out_avals)
        out_names = tuple(self.out_names)
        pname = self.partition_name

        def _body(*args):
            operands = list(args)
            if pname is not None:
                operands.append(partition_id_tensor())
            return tuple(
                _bass_exec_p.bind(
                    *operands,
                    out_avals=out_avals,
                    in_names=tuple(all_in_names),
                    out_names=out_names,
                    lowering_input_output_aliases=(),
                    sim_require_finite=True,
                    sim_require_nnan=True,
                    nc=nc,
                )
            )

        donate = tuple(range(n_params, n_params + n_outs))
        if self.n_cores == 1:
            return jax.jit(_body, donate_argnums=donate, keep_unused=True)
        mesh = Mesh(np.asarray(self.devices), ("core",))
        in_specs = (PartitionSpec("core"),) * (n_params + n_outs)
        out_specs = (PartitionSpec("core"),) * n_outs
        return jax.jit(
            shard_map(
                _body, mesh=mesh, in_specs=in_specs, out_specs=out_specs,
                check_rep=False,
            ),
            donate_argnums=donate,
            keep_unused=True,
        )

    def put_inputs(self, in_maps):
        import jax
        from jax.sharding import Mesh, PartitionSpec, NamedSharding

        args = []
        for name in self.in_names:
            per = [np.asarray(in_maps[c][name]) for c in range(self.n_cores)]
            args.append(per[0] if self.n_cores == 1 else np.concatenate(per, 0))
        if self.n_cores == 1:
            return [jax.device_put(a, self.devices[0]) for a in args]
        mesh = Mesh(np.asarray(self.devices), ("core",))
        sh = NamedSharding(mesh, PartitionSpec("core"))
        return [jax.device_put(a, sh) for a in args]

    def _dev_zeros(self):
        import jax
        import jax.numpy as jnp
        from jax.sharding import Mesh, PartitionSpec, NamedSharding

        if self.n_cores == 1:
            dev = self.devices[0]
            return [
                jax.device_put(jnp.zeros(a.shape, a.dtype), dev)
                for a in self.out_avals
            ]
        mesh = Mesh(np.asarray(self.devices), ("core",))
        sh = NamedSharding(mesh, PartitionSpec("core"))
        outs = []
        for a in self.out_avals:
            shape = (self.n_cores * a.shape[0], *a.shape[1:])
            outs.append(jax.jit(
                lambda shape=shape, dt=a.dtype: jnp.zeros(shape, dt),
                out_shardings=sh)())
        return outs

    def run_device(self, dev_args):
        jax = self.jax
        if self._fn is None:
            self._fn = self._build_fn()
        outs = self._fn(*dev_args, *self._dev_zeros())
        jax.block_until_ready(outs)
        return outs

    def run(self, in_maps):
        outs = self.run_device(self.put_inputs(in_maps))
        res = []
        for c in range(self.n_cores):
            m = {}
            for i, name in enumerate(self.out_names):
                a = np.asarray(outs[i])
                if self.n_cores > 1:
                    a = a.reshape(self.n_cores, *self.out_avals[i].shape)[c]
                m[name] = a
            res.append(m)
        return res


# --------------------------------------------------------------------------
def _build(with_bq):
    nc = bacc.Bacc(
        "TRN2", target_bir_lowering=False, debug=False, num_swdge_queues=NQ
    )
    z_ext = nc.dram_tensor("z_ext", [N + 1, D], _bf16, kind="ExternalInput")
    zcT = nc.dram_tensor("zcT", [P, NPAD], _bf16, kind="ExternalInput")
    idx_lo = nc.dram_tensor("idx_lo", [P, TILES * 128], _i16, kind="ExternalInput")
    idx_hi = nc.dram_tensor("idx_hi", [P, TILES * 128], _i16, kind="ExternalInput")
    mt_d = nc.dram_tensor("Mt", [P, D], _bf16, kind="ExternalInput")
    wv_d = nc.dram_tensor("Wv", [P, D], _bf16, kind="ExternalInput")
    id_d = nc.dram_tensor("ident", [P, P], _bf16, kind="ExternalInput")
    bv_d = nc.dram_tensor("bv", [P, 1], _f32, kind="ExternalInput")
    if with_bq:
        m0_d = nc.dram_tensor("m0rep", [P, D], _f32, kind="ExternalInput")
    hT_d = nc.dram_tensor("hT", [P, NPAD], _f32, kind="ExternalOutput")
    al_d = nc.dram_tensor("alpha", [NPAD, K], _f32, kind="ExternalOutput")

    with tile.TileContext(nc) as tc:
        with (
            tc.tile_pool(name="const", bufs=1) as cpool,
            tc.tile_pool(name="glo", bufs=3) as glop,
            tc.tile_pool(name="ghi", bufs=3) as ghip,
            tc.tile_pool(name="g", bufs=3) as gp,
            tc.tile_pool(name="scr", bufs=2) as scrp,
            tc.tile_pool(name="sg", bufs=2) as sgp,
            tc.tile_pool(name="qk", bufs=3) as qkp,
            tc.tile_pool(name="small", bufs=6) as smp,
            tc.tile_pool(name="outs", bufs=3) as outp,
            tc.tile_pool(name="ps_qk", bufs=2, space="PSUM") as ps_qk,
            tc.tile_pool(name="ps_agg", bufs=2, space="PSUM") as ps_agg,
            tc.tile_pool(name="ps_ht", bufs=2, space="PSUM") as ps_ht,
        ):
            mt = cpool.tile([P, D], _bf16)
            nc.sync.dma_start(out=mt[:], in_=mt_d[:])
            wv = cpool.tile([P, D], _bf16)
            nc.sync.dma_start(out=wv[:], in_=wv_d[:])
            ident = cpool.tile([P, P], _bf16)
            nc.sync.dma_start(out=ident[:], in_=id_d[:])
            bv = cpool.tile([P, 1], _f32)
            nc.sync.dma_start(out=bv[:], in_=bv_d[:])
            if with_bq:
                m0 = cpool.tile([P, D], _f32)
                nc.sync.dma_start(out=m0[:], in_=m0_d[:])
            zct = cpool.tile([P, NPAD], _bf16)
            nc.sync.dma_start(out=zct[:], in_=zcT[:])
            ilo = cpool.tile([P, TILES * 128], _i16)
            nc.sync.dma_start(out=ilo[:], in_=idx_lo[:])
            ihi = cpool.tile([P, TILES * 128], _i16)
            nc.sync.dma_start(out=ihi[:], in_=idx_hi[:])

            z_lo = z_ext[:, :]
            z_hi = z_ext[HIBASE:, :]

            qi = 0
            for t in range(TILES):
                glo = glop.tile([P, K, D], _bf16)
                ghi = ghip.tile([P, K, D], _bf16)
                for h in range(2):
                    nc.gpsimd.dma_gather(
                        out_ap=glo[:, h * 8:(h + 1) * 8, :],
                        in_ap=z_lo,
                        idxs_ap=ilo[:, t * 128 + h * 64: t * 128 + (h + 1) * 64],
                        num_idxs=1024, num_idxs_reg=1024, elem_size=D,
                        queue_num=qi % NQ,
                    )
                    qi += 1
                for h in range(2):
                    nc.gpsimd.dma_gather(
                        out_ap=ghi[:, h * 8:(h + 1) * 8, :],
                        in_ap=z_hi,
                        idxs_ap=ihi[:, t * 128 + h * 64: t * 128 + (h + 1) * 64],
                        num_idxs=1024, num_idxs_reg=1024, elem_size=D,
                        queue_num=qi % NQ,
                    )
                    qi += 1

                # combine the two gather passes (one of each pair is zeros)
                g = gp.tile([P, K, D], _bf16)
                nc.vector.tensor_tensor(
                    out=g[:, :, :], in0=glo[:, :, :], in1=ghi[:, :, :],
                    op=mybir.AluOpType.add,
                )

                # qk = z_c @ Mt  (row-major: lhsT = zcT slice)
                qk_ps = ps_qk.tile([P, D], _f32)
                nc.tensor.matmul(
                    out=qk_ps[:], lhsT=zct[:, t * P:(t + 1) * P], rhs=mt[:],
                    start=True, stop=True,
                )
                qk = qkp.tile([P, D], _bf16)
                if with_bq:
                    nc.vector.tensor_tensor(
                        out=qk[:], in0=qk_ps[:], in1=m0[:],
                        op=mybir.AluOpType.add,
                    )
                else:
                    nc.scalar.activation(
                        out=qk[:], in_=qk_ps[:],
                        func=mybir.ActivationFunctionType.Copy,
                    )

                # e[n,j] = sum_d g[n,j,d] * qk[n,d]
                qk_b = bass.AP(
                    tensor=qk[:].tensor, offset=qk[:].offset,
                    ap=[qk[:].ap[0], [0, K], qk[:].ap[1]],
                )
                scr = scrp.tile([P, K, D], _bf16)
                nc.vector.tensor_tensor(
                    out=scr[:, :, :], in0=g[:, :, :], in1=qk_b,
                    op=mybir.AluOpType.mult,
                )
                e = smp.tile([P, K], _f32)
                nc.vector.tensor_reduce(
                    out=e[:], in_=scr[:, :, :], axis=mybir.AxisListType.X,
                    op=mybir.AluOpType.add,
                )

                # softmax over j (fp32)
                mx = smp.tile([P, 1], _f32)
                nc.vector.tensor_reduce(
                    out=mx[:], in_=e[:], axis=mybir.AxisListType.X,
                    op=mybir.AluOpType.max,
                )
                es = smp.tile([P, K], _f32)
                nc.vector.tensor_scalar_sub(out=es[:], in0=e[:], scalar1=mx[:])
                pex = smp.tile([P, K], _f32)
                nc.scalar.activation(
                    out=pex[:], in_=es[:], func=mybir.ActivationFunctionType.Exp,
                )
                s = smp.tile([P, 1], _f32)
                nc.vector.tensor_reduce(
                    out=s[:], in_=pex[:], axis=mybir.AxisListType.X,
                    op=mybir.AluOpType.add,
                )
                rinv = smp.tile([P, 1], _f32)
                nc.vector.reciprocal(out=rinv[:], in_=s[:])
                alph = outp.tile([P, K], _f32)
                nc.vector.tensor_scalar_mul(out=alph[:], in0=pex[:], scalar1=rinv[:])

                # aggregation: aggT = sum_j (alpha_j * G_j)^T  via PE transposes
                sg = sgp.tile([P, K, D], _bf16)
                for j in range(K):
                    nc.vector.tensor_scalar_mul(
                        out=sg[:, j, :], in0=g[:, j, :], scalar1=alph[:, j:j + 1],
                    )
                agg_ps = ps_agg.tile([P, P], _f32)
                for j in range(K):
                    nc.tensor.matmul(
                        out=agg_ps[:], lhsT=sg[:, j, :], rhs=ident[:],
                        start=(j == 0), stop=(j == K - 1),
                    )
                aggsb = qkp.tile([P, P], _bf16)
                nc.scalar.activation(
                    out=aggsb[:], in_=agg_ps[:],
                    func=mybir.ActivationFunctionType.Copy,
                )

                # hT = Wv^T @ aggT  (+ bv per-partition)
                ht_ps = ps_ht.tile([P, P], _f32)
                nc.tensor.matmul(
                    out=ht_ps[:], lhsT=wv[:], rhs=aggsb[:], start=True, stop=True,
                )
                ht = outp.tile([P, P], _f32)
                nc.scalar.activation(
                    out=ht[:], in_=ht_ps[:],
                    func=mybir.ActivationFunctionType.Identity, bias=bv[:],
                )

                nc.sync.dma_start(out=hT_d[:, t * P:(t + 1) * P], in_=ht[:])
                nc.sync.dma_start(out=al_d[t * P:(t + 1) * P, :], in_=alph[:])

    nc.compile()
    _split_multiwait(nc)
    return nc


def _wrap_idx(arr):
    """[TILES, 2048] edge-ordered (j-major) int16 -> [128, TILES*128] wrapped
    layout: per 1024-idx instruction, idx i lives at [i%16 (+16r), i//16]."""
    T = arr.shape[0]
    a = arr.reshape(T, 2, 64, 16).transpose(0, 1, 3, 2)     # [T, 2, 16, 64]
    a = np.tile(a, (1, 1, 8, 1))                            # [T, 2, 128, 64]
    return np.ascontiguousarray(
        a.transpose(2, 0, 1, 3).reshape(P, T * 128)
    )


def kernel(z, src, Wq, bq, Wk, bk, Wv, bv, Ws1, bs1, Ws2, bs2):
    z = np.asarray(z, dtype=np.float32)
    src = np.asarray(src).astype(np.int64)
    Wq = np.asarray(Wq, dtype=np.float32)
    Wk = np.asarray(Wk, dtype=np.float32)
    Wv_ = np.asarray(Wv, dtype=np.float32)
    bq = np.asarray(bq, dtype=np.float32)
    bv_ = np.asarray(bv, dtype=np.float32)

    bf = ml_dtypes.bfloat16
    Mt = (TAU * (Wq @ Wk.T)).astype(bf)
    m0 = (TAU * (bq @ Wk.T)).astype(np.float32)          # [D]
    with_bq = bool(np.any(m0))

    # gather table with a zeros row spliced in at ZROW
    z_ext = np.empty((N + 1, D), dtype=bf)
    z_ext[:ZROW] = z[:ZROW].astype(bf)
    z_ext[ZROW] = 0
    z_ext[ZROW + 1:] = z[ZROW:].astype(bf)

    key = with_bq
    if key not in _cache:
        nc = _build(with_bq)
        _cache[key] = (nc, _Runner(nc, NCORES))
    nc, runner = _cache[key]

    ident = np.eye(P, dtype=bf)
    bv_col = bv_.reshape(P, 1)
    Wv_b = Wv_.astype(bf)
    m0rep = np.tile(m0.reshape(1, D), (P, 1)).astype(np.float32)

    mapped = src + (src >= ZROW)                         # [N, K] rows in z_ext
    in_maps = []
    for c in range(NCORES):
        rows = np.arange(c * NPC, (c + 1) * NPC)
        zc = np.zeros((NPAD, D), np.float32)
        zc[:NPC] = z[rows]
        zcT = np.ascontiguousarray(zc.T).astype(bf)      # [128, NPAD]

        mc = np.zeros((NPAD, K), np.int64)
        mc[:NPC] = mapped[rows]
        lo = np.where(mc <= 32767, mc, ZROW).astype(np.int16)
        hi = np.where(mc >= 32768, mc - HIBASE, ZROW - HIBASE).astype(np.int16)
        # edge order per tile: slot i = j*128 + n  (j-major)
        lo_t = lo.reshape(TILES, P, K).transpose(0, 2, 1).reshape(TILES, K * P)
        hi_t = hi.reshape(TILES, P, K).transpose(0, 2, 1).reshape(TILES, K * P)

        m_ = {
            "z_ext": z_ext,
            "zcT": zcT,
            "idx_lo": _wrap_idx(lo_t),
            "idx_hi": _wrap_idx(hi_t),
            "Mt": Mt,
            "Wv": Wv_b,
            "ident": ident,
            "bv": bv_col,
        }
        if with_bq:
            m_["m0rep"] = m0rep
        in_maps.append(m_)

    kernel._last_runner = runner
    kernel._last_in_maps = in_maps
    res = runner.run(in_maps)

    h = np.empty((N, D), np.float32)
    alpha = np.empty((N, K), np.float32)
    for c in range(NCORES):
        h[c * NPC:(c + 1) * NPC] = res[c]["hT"][:, :NPC].T
        alpha[c * NPC:(c + 1) * NPC] = res[c]["alpha"][:NPC]
    return h, alpha
